# revision 12
# baseline (speedup 1.0000x reference)
"""Multivariate Hawkes log-likelihood on 8 Trainium2 NeuronCores.

Math (per core; blocks of 128 sorted events, core c owns blocks 4c..4c+3 for
term1 and grid columns [13c, 13c+13) for term2):

  vh[j, b*64+s'] = [sp_jb == s'] * exp(beta*(t_jb - tref_b))     (bf16, DVE
      is_equal + mult with stride-0 broadcast APs; no 1MB onehot DMA)
  W[b, s']  = sum_j vh[j, b, s']            (4 ones-reduce matmuls, N=512)
  A-part    = sum_{b'<myb_k} D[b',k] W[b',s']  folded as ONE seed matmul over
      vd[b', (k,s')] = Dmy[b',k]*W[b',s']   (DVE broadcast mult, K=16 x2)
  in-block  = ONE matmul (lmask [128,128] @ vh_my [128, 4*64])
  lam_i     = mu[sp_i] + umy_i * (in-block + A-part) . alpha_row_i
  term1     = sum log lam   (relu dropped: mu>=0.1 and excitation >=0)

  term2 C[g,s'] = sum_{b full} exp(-beta(tg-tref_b)) W[b,s']   (2 matmuls K=16)
                + sum_{j in boundary blk} [sp_j==s'] exp(beta(t_j-tg))  (z via
      ones-reduce, flat->DRAM->[13,64] scatter)
  term2 = sum_g,s' gscale_g * (mu_s' + C[g,s'])   (relu dropped, same reason)

Single-sync-wait discipline (this walrus build allows ONE fresh cross-engine
wait per instruction): inputs ride in one f32 blob (bf16 consts bitcast
inside), each engine touches the blob once; producers are routed so every
consumer joins at most one fresh engine; PSUM pools sized so bank-reuse WAR
waits share the semaphore of an existing data wait; DMA count kept at 8.
"""

import numpy as np

import concourse.bass as bass
import concourse.mybir as mybir
import concourse.tile as tile
from concourse.bass_utils import run_bass_kernel_spmd
from concourse.vector_clock import ScopedClock, VectorClock


def _split_drain_and_barrier(self, tick_clock, wait_clock):
    gclock = tick_clock.global_clock
    for proc in range(len(gclock)):
        tick = gclock[proc]
        if tick <= 0:
            continue
        vc1 = VectorClock()
        vc1.require_at_least(proc, tick)
        di = self.nc.sync.drain()
        wait_clock.add_sem_waits(di.ins, ScopedClock({None: vc1}))
    self.nc.all_engine_barrier()
    assert self.sems is not None
    popped = self.nc._tile_sem_poison_stack.pop()
    assert popped is self._sem_poison
    self.nc.clear_and_free_semaphores(list(self.sems.allocated().values()))
    self.nc.all_engine_barrier()


tile.TileContext._drain_and_barrier = _split_drain_and_barrier

N, NB, BS, S, G, GS = 4096, 32, 128, 64, 100, 13
T0, T1, INT_RES = 0.0, 100.0, 100
NBLK = 4
BIG = 1.0e5
F32 = mybir.dt.float32
BF16 = mybir.dt.bfloat16
AF = mybir.ActivationFunctionType
ALU = mybir.AluOpType

# f32 blob fields (rows, cols)
_FIELDS = [
    ("iota", BS, S), ("spf", BS, NB), ("tdel", BS, NB),
    ("spmy", BS, NBLK), ("tdelmy", BS, NBLK), ("mugmy", BS, NBLK),
    ("alphag", BS, NBLK * S), ("spbnd", BS, GS), ("targ", BS, GS),
    ("mu64", S, 1), ("gsc64", S, GS), ("ones_f", BS, 1), ("ones64", S, 1),
    ("dmy8", 16, 2 * NBLK),
    # bf16 region, bitcast: layout in bf16 cols (even f32-col aligned):
    #   ones_b [128, 0:2], lmask_b [128, 2:130], onesM_b [16, 130:258],
    #   d2a_b [16, 258:272], d2b_b [16, 272:286], alphaT_b [64, 286:350]
    ("bfreg", BS, 176),
]
_OFF = {}
_cur = 0
for _nm, _r, _c in _FIELDS:
    _OFF[_nm] = _cur
    _cur += _c
BLOB_COLS = _cur

_CACHE = {}


def _build_program():
    nc = bass.Bass()
    blob = nc.dram_tensor("blob", [BS, BLOB_COLS], F32, kind="ExternalInput")
    outd = nc.dram_tensor("outd", [1, 2], F32, kind="ExternalOutput")
    scr_v = nc.dram_tensor("scr_v", [1, 1024], BF16, kind="Internal")
    scr_s = nc.dram_tensor("scr_s", [1, 1024], BF16, kind="Internal")
    zscr = nc.dram_tensor("zscr", [1, GS * S], BF16, kind="Internal")

    with tile.TileContext(nc) as tc:
        with (
            tc.tile_pool(name="const", bufs=1) as cp,
            tc.tile_pool(name="psw", bufs=1, space=bass.MemorySpace.PSUM) as pw,
            tc.tile_pool(name="psz", bufs=1, space=bass.MemorySpace.PSUM) as pz,
            tc.tile_pool(name="psr", bufs=1, space=bass.MemorySpace.PSUM) as pr,
            tc.tile_pool(name="pst", bufs=1, space=bass.MemorySpace.PSUM) as pt,
        ):
            bsb = cp.tile([BS, BLOB_COLS], F32, tag="bsb")
            nc.sync.dma_start(bsb[0:64, :], blob[0:64, :])
            nc.sync.dma_start(bsb[64:128, :], blob[64:128, :])

            def fld(name, rows=None):
                r = dict((n, (rr, cc)) for n, rr, cc in _FIELDS)[name]
                off = _OFF[name]
                return bsb[0:(rows or r[0]), off:off + r[1]]

            bfr = fld("bfreg").bitcast(BF16)
            ones_b = bfr[:, 0:1]
            lmask_b = bfr[:, 2:130]
            onesM_b = bfr[0:16, 130:258]
            d2a_b = bfr[0:16, 258:258 + GS]
            d2b_b = bfr[0:16, 272:272 + GS]
            alphaT_b = bfr[0:S, 286:286 + S]

            # --- touches: absorb both blob-DMA semaphores per engine ---
            ptch = pw.tile([1, 1], F32, tag="pw", bufs=3, name="ptch")
            nc.tensor.matmul(ptch[:], bsb[0:1, 0:1], bsb[0:1, 0:1],
                             start=True, stop=True)
            ptch2 = pw.tile([1, 1], F32, tag="pw", bufs=3, name="ptch2")
            nc.tensor.matmul(ptch2[:], bsb[64:65, 0:1], bsb[64:65, 0:1],
                             start=True, stop=True)
            dvetch = cp.tile([1, 2], F32, tag="dvetch")
            nc.vector.tensor_copy(dvetch[0:1, 0:1], bsb[0:1, 0:1])
            nc.vector.tensor_copy(dvetch[0:1, 1:2], bsb[64:65, 0:1])
            # mark ptch/ptch2 read by vector so later pt-ring reuse carries a
            # dominated vector WAR instead of a PE WAW self-wait
            ptchr = cp.tile([1, 2], F32, tag="ptchr")
            nc.vector.tensor_copy(ptchr[0:1, 0:1], ptch[:])
            nc.vector.tensor_copy(ptchr[0:1, 1:2], ptch2[:])
            acttch = cp.tile([1, 2], F32, tag="acttch")
            nc.scalar.copy(acttch[0:1, 0:1], bsb[0:1, 0:1])
            nc.scalar.copy(acttch[0:1, 1:2], bsb[64:65, 0:1])

            # --- ACT preamble: exps ---
            v_t = cp.tile([BS, NB], F32, tag="v_t")
            nc.scalar.activation(v_t[:], fld("tdel"), AF.Exp)
            vmy = cp.tile([BS, NBLK], F32, tag="vmy")
            nc.scalar.activation(vmy[:], fld("tdelmy"), AF.Exp)
            umy = cp.tile([BS, NBLK], F32, tag="umy")
            nc.scalar.activation(umy[:], fld("tdelmy"), AF.Exp, scale=-1.0)
            vmp = cp.tile([BS, GS], F32, tag="vmp")
            nc.scalar.activation(vmp[:], fld("targ"), AF.Exp)
            # vector absorbs the full ACT preamble clock (vmp is last), and
            # re-exports the operands gpsimd needs, so gpsimd only ever joins
            # vector (gpsimd touch copies cost 1-4us, so none are used)
            dvetch2 = cp.tile([1, 1], F32, tag="dvetch2")
            nc.vector.tensor_copy(dvetch2[:], vmp[0:1, 0:1])
            v16b = cp.tile([BS, 16], F32, tag="v16b")
            nc.vector.tensor_copy(v16b[:], v_t[:, 16:32])
            vmpv = cp.tile([BS, GS], F32, tag="vmpv")
            nc.vector.tensor_copy(vmpv[:], vmp[:])
            vmyv = cp.tile([BS, NBLK], F32, tag="vmyv")
            nc.vector.tensor_copy(vmyv[:], vmy[:])
            umyv = cp.tile([BS, NBLK], F32, tag="umyv")
            nc.vector.tensor_copy(umyv[:], umy[:])

            # --- DVE builds (bf16) ---
            iota2 = fld("iota").unsqueeze(1)
            oh = cp.tile([BS, NB * S], BF16, tag="oh")
            vh = cp.tile([BS, NB * S], BF16, tag="vh")

            def iseq(out3, spcols, nblk):
                nc.vector.tensor_tensor(
                    out3, iota2.broadcast_to([BS, nblk, S]),
                    spcols.unsqueeze(2).broadcast_to([BS, nblk, S]),
                    ALU.is_equal)

            # ohB, ohA, ohz first (is_equal is vector-only); gpsimd
            # consumes them for z while vector continues with vhA/vhB
            iseq(oh[:, 1024:2048].rearrange("p (b s) -> p b s", b=16),
                 bsb[0:BS, _OFF["spf"] + 16:_OFF["spf"] + 32], 16)
            iseq(oh[:, 0:1024].rearrange("p (b s) -> p b s", b=16),
                 bsb[0:BS, _OFF["spf"]:_OFF["spf"] + 16], 16)
            ohz = cp.tile([BS, GS * S], BF16, tag="ohz")
            iseq(ohz[:].rearrange("p (g s) -> p g s", g=GS), fld("spbnd"), GS)
            # gpsimd: z = ohz * vmp (term2 arm, off the W critical path)
            z = cp.tile([BS, GS * S], BF16, tag="z")
            nc.gpsimd.tensor_tensor(
                z[:].rearrange("p (g s) -> p g s", g=GS),
                ohz[:].rearrange("p (g s) -> p g s", g=GS),
                vmpv[:].unsqueeze(2).broadcast_to([BS, GS, S]), ALU.mult)
            # vector: vhA then vhB (gpsimd latency is too unreliable for the
            # W critical path)
            nc.vector.tensor_tensor(
                vh[:, 0:1024].rearrange("p (b s) -> p b s", b=16),
                oh[:, 0:1024].rearrange("p (b s) -> p b s", b=16),
                v_t[:, 0:16].unsqueeze(2).broadcast_to([BS, 16, S]), ALU.mult)
            nc.vector.tensor_tensor(
                vh[:, 1024:2048].rearrange("p (b s) -> p b s", b=16),
                oh[:, 1024:2048].rearrange("p (b s) -> p b s", b=16),
                v16b[:].unsqueeze(2).broadcast_to([BS, 16, S]), ALU.mult)
            # my-blocks one-hot on vector; vh_my mult on gpsimd
            ohmy = cp.tile([BS, NBLK * S], BF16, tag="ohmy")
            iseq(ohmy[:].rearrange("p (b s) -> p b s", b=NBLK), fld("spmy"),
                 NBLK)
            vhmy = cp.tile([BS, NBLK * S], BF16, tag="vhmy")
            nc.gpsimd.tensor_tensor(
                vhmy[:].rearrange("p (b s) -> p b s", b=NBLK),
                ohmy[:].rearrange("p (b s) -> p b s", b=NBLK),
                vmyv[:].unsqueeze(2).broadcast_to([BS, NBLK, S]), ALU.mult)
            # --- term1 in-block matmul (starts psr accumulation group) ---
            psr = pr.tile([BS, NBLK * S], F32, tag="pr")
            nc.tensor.matmul(psr[:], lmask_b, vhmy[:], start=True, stop=False)

            # --- W ones-reduce (4 matmuls, bf16 N=512), copy right after
            # each matmul so the emitted PE-clock wait is minimal ---
            wflat_v = cp.tile([1, 1024], BF16, tag="wflat_v")
            wflat_s = cp.tile([1, 1024], BF16, tag="wflat_s")
            for c4 in range(4):
                pmw = pw.tile([1, 512], F32, tag="pw", bufs=3, name=f"pw{c4}")
                nc.tensor.matmul(pmw[:], ones_b,
                                 vh[:, c4 * 512:(c4 + 1) * 512],
                                 start=True, stop=True)
                if c4 == 0:
                    nc.vector.tensor_copy(wflat_v[0:1, 0:512], pmw[:])
                elif c4 == 1:
                    nc.vector.tensor_copy(wflat_v[0:1, 512:1024], pmw[:])
                elif c4 == 2:
                    nc.scalar.copy(wflat_s[0:1, 0:512], pmw[:])
                else:
                    nc.scalar.copy(wflat_s[0:1, 512:1024], pmw[:])
                if c4 == 1:
                    nc.sync.dma_start(scr_v[:], wflat_v[:])
                if c4 == 3:
                    nc.scalar.dma_start(scr_s[:], wflat_s[:])

            # --- z ones-reduce (2 matmuls) ---
            psz0 = pz.tile([1, 512], F32, tag="pz", bufs=1, name="psz0")
            nc.tensor.matmul(psz0[:], ones_b, z[:, 0:512], start=True,
                             stop=True)
            zflat = cp.tile([1, GS * S], BF16, tag="zflat")
            nc.scalar.copy(zflat[0:1, 0:512], psz0[:])
            psz1 = pz.tile([1, GS * S - 512], F32, tag="pz", bufs=1,
                           name="psz1")
            nc.tensor.matmul(psz1[:], ones_b, z[:, 512:GS * S], start=True,
                             stop=True)
            nc.scalar.copy(zflat[0:1, 512:GS * S], psz1[:])
            nc.sync.dma_start(zscr[:], zflat[:])

            # --- DRAM roundtrips back in: W -> wsb halves, z -> pzT ---
            wsb1 = cp.tile([16, S], BF16, tag="wsb1")
            wsb2 = cp.tile([16, S], BF16, tag="wsb2")
            nc.sync.dma_start(wsb1[:],
                              scr_v[:].rearrange("p (r s) -> (p r) s", r=16))
            nc.scalar.dma_start(wsb2[:],
                                scr_s[:].rearrange("p (r s) -> (p r) s", r=16))
            # pzT[s', g] = zflat[g*64+s']  (transposed scatter from DRAM)
            pzT = cp.tile([S, GS], BF16, tag="pzT")
            nc.sync.dma_start(pzT[:],
                              zscr[:].rearrange("p (g s) -> (p s) g", g=GS))

            # --- vd + seed matmuls (A-part of term1) ---
            vd1 = cp.tile([16, NBLK * S], BF16, tag="vd1")
            nc.vector.tensor_tensor(
                vd1[:].rearrange("p (k s) -> p k s", k=NBLK),
                bsb[0:16, _OFF["dmy8"]:_OFF["dmy8"] + 4].unsqueeze(2)
                    .broadcast_to([16, NBLK, S]),
                wsb1[:].unsqueeze(1).broadcast_to([16, NBLK, S]), ALU.mult)
            vd2 = cp.tile([16, NBLK * S], BF16, tag="vd2")
            nc.vector.tensor_tensor(
                vd2[:].rearrange("p (k s) -> p k s", k=NBLK),
                bsb[0:16, _OFF["dmy8"] + 4:_OFF["dmy8"] + 8].unsqueeze(2)
                    .broadcast_to([16, NBLK, S]),
                wsb2[:].unsqueeze(1).broadcast_to([16, NBLK, S]), ALU.mult)
            nc.tensor.matmul(psr[:], onesM_b, vd1[:], start=False, stop=False,
                             skip_group_check=True)
            nc.tensor.matmul(psr[:], onesM_b, vd2[:], start=False, stop=True,
                             skip_group_check=True)

            # --- D2 full-block part of term2, transposed: [64, 13] ---
            pst2T = pt.tile([S, GS], F32, tag="pt", bufs=3, name="pst2T")
            nc.tensor.matmul(pst2T[:], wsb1[:], d2a_b, start=True, stop=False)
            nc.tensor.matmul(pst2T[:], wsb2[:], d2b_b, start=False, stop=True)

            # ct1 copy first: absorbs the latest PE tick into vector's
            # clock so junk's psr read needs no fresh PE wait
            ct1 = cp.tile([S, GS], BF16, tag="ct1")
            nc.vector.tensor_copy(ct1[:], pst2T[:])

            # --- term1 tail ---
            # psr -> SBUF first: the copy carries the single fresh PE wait,
            # the rest are DVE-only joins. vp folds the per-event decay umy;
            # junk then multiplies the alpha rows straight from the blob.
            psrsb = cp.tile([BS, NBLK * S], F32, tag="psrsb")
            nc.vector.tensor_copy(psrsb[:], psr[:])
            vp = cp.tile([BS, NBLK * S], F32, tag="vp")
            nc.vector.tensor_tensor(
                vp[:].rearrange("p (k s) -> p k s", k=NBLK),
                psrsb[:].rearrange("p (k s) -> p k s", k=NBLK),
                umyv[:].unsqueeze(2).broadcast_to([BS, NBLK, S]), ALU.mult)
            junk = cp.tile([BS, NBLK * S], F32, tag="junk")
            nc.vector.tensor_tensor(junk[:], vp[:], fld("alphag"), ALU.mult)
            red4 = cp.tile([BS, NBLK], F32, tag="red4")
            nc.vector.reduce_sum(
                red4[:], junk[:].rearrange("p (k s) -> p k s", k=NBLK),
                mybir.AxisListType.X)
            lam4 = cp.tile([BS, NBLK], F32, tag="lam4")
            nc.vector.tensor_tensor(lam4[:], red4[:], fld("mugmy"), ALU.add)
            logacc = cp.tile([BS, NBLK], F32, tag="logacc")
            nc.scalar.activation(logacc[:], lam4[:], AF.Ln)
            t1red = cp.tile([BS, 1], F32, tag="t1red")
            nc.vector.reduce_sum(t1red[:], logacc[:], mybir.AxisListType.X)
            ps_s1 = pt.tile([1, 1], F32, tag="pt", bufs=3, name="ps_s1")
            nc.tensor.matmul(ps_s1[:], t1red[:], fld("ones_f"), start=True,
                             stop=True)

            # --- term2 tail: val = alphaT @ (pst2T + pzT), then scale+sum ---
            psval = pt.tile([S, GS], F32, tag="pt", bufs=3, name="psval")
            nc.tensor.matmul(psval[:], alphaT_b, ct1[:], start=True,
                             stop=False)
            nc.tensor.matmul(psval[:], alphaT_b, pzT[:], start=False,
                             stop=True)
            t2b = cp.tile([S, GS], F32, tag="t2b")
            nc.vector.tensor_scalar_add(t2b[:], psval[:], fld("mu64"))
            t2c = cp.tile([S, GS], F32, tag="t2c")
            nc.vector.tensor_tensor(t2c[:], t2b[:], fld("gsc64"), ALU.mult)
            t2red = cp.tile([S, 1], F32, tag="t2red")
            nc.vector.reduce_sum(t2red[:], t2c[:], mybir.AxisListType.X)
            ps_s2 = pt.tile([1, 1], F32, tag="pt", bufs=3, name="ps_s2")
            nc.tensor.matmul(ps_s2[:], t2red[:], fld("ones64"), start=True,
                             stop=True)

            out_sb = cp.tile([1, 2], F32, tag="out_sb")
            nc.scalar.copy(out_sb[0:1, 0:1], ps_s1[:])
            nc.scalar.copy(out_sb[0:1, 1:2], ps_s2[:])
            nc.scalar.dma_start(outd[:], out_sb[:])

    return nc


def _to_bf16_bits(x):
    xb = np.asarray(x, np.float32).view(np.uint32)
    return ((xb + 0x8000) >> 16).astype(np.uint16)


def _host_inputs(data, mu_param, alpha, beta):
    times = np.ascontiguousarray(np.asarray(data)[:, 0], dtype=np.float64)
    sp = np.asarray(data)[:, 1].astype(np.int32)
    beta = float(beta)
    mu = np.asarray(mu_param, dtype=np.float32)
    alpha_f = np.asarray(alpha, dtype=np.float32) * np.float32(beta)

    tref = times[::BS]                                       # [32]
    spf = sp.reshape(NB, BS).T.astype(np.float32)            # [128, 32]
    tdel = ((times.reshape(NB, BS).T - tref[None, :]) * beta).astype(np.float32)
    iota = np.broadcast_to(np.arange(S, dtype=np.float32), (BS, S))
    lmask = (np.arange(BS)[:, None] < np.arange(BS)[None, :]).astype(np.float32)
    tgrid = np.linspace(T0, T1, INT_RES)                     # [100] f64
    cuts = np.searchsorted(times, tgrid, side="left")        # events with t<tg

    # bf16 const region (shared across cores)
    bfbits = np.zeros((BS, 352), np.uint16)
    bfbits[:, 0:2] = _to_bf16_bits(np.ones((BS, 2), np.float32))
    bfbits[:, 2:130] = _to_bf16_bits(lmask)
    bfbits[0:16, 130:258] = _to_bf16_bits(np.ones((16, 128), np.float32))

    in_maps = []
    for c in range(8):
        myb = [4 * c + k for k in range(NBLK)]
        spmy = spf[:, myb]
        tdelmy = tdel[:, myb]
        rows = sp.reshape(NB, BS)[myb]                      # [4, 128]
        alphag = alpha_f[rows.reshape(-1)].reshape(NBLK, BS, S) \
            .transpose(1, 0, 2).reshape(BS, NBLK * S)
        mugmy = mu[rows].T.astype(np.float32)               # [128, 4]

        # D for my blocks: Dmy[b', k] = exp(-beta(tref_myk - tref_b')), b'<myk
        bidx = np.arange(NB)
        dmy = np.zeros((NB, NBLK), np.float64)
        for k, b in enumerate(myb):
            m = bidx < b
            dmy[m, k] = np.exp(-beta * (tref[b] - tref[m]))
        dmy8 = np.concatenate([dmy[0:16], dmy[16:32]], axis=1) \
            .astype(np.float32)                              # [16, 8]

        # term2 grid columns for this core
        spbnd = np.zeros((BS, GS), np.float32)
        targ = np.full((BS, GS), -BIG, np.float32)
        d2 = np.zeros((NB, GS), np.float64)
        gsc = np.zeros((GS, 1), np.float32)
        for i in range(GS):
            g = c * GS + i
            if g >= G:
                continue
            tg = tgrid[g]
            gsc[i, 0] = np.float32(T1 / INT_RES)
            cut = cuts[g]
            nfull = cut // BS
            d2[0:nfull, i] = np.exp(-beta * (tg - tref[0:nfull]))
            rem = cut % BS
            if rem > 0:
                bg = nfull
                spbnd[:, i] = spf[:, bg]
                tb = times[bg * BS:(bg + 1) * BS]
                targ[0:rem, i] = (beta * (tb[0:rem] - tg)).astype(np.float32)
        d2 = d2.astype(np.float32)

        bfb = bfbits.copy()
        bfb[0:16, 258:258 + GS] = _to_bf16_bits(d2[0:16])
        bfb[0:16, 272:272 + GS] = _to_bf16_bits(d2[16:32])
        bfb[0:S, 286:286 + S] = _to_bf16_bits(alpha_f.T)
        u32 = (bfb[:, 1::2].astype(np.uint32) << 16) | bfb[:, 0::2]
        bfreg = u32.view(np.float32)

        vals = {
            "iota": iota, "spf": spf, "tdel": tdel, "spmy": spmy,
            "tdelmy": tdelmy, "mugmy": mugmy, "alphag": alphag,
            "spbnd": spbnd, "targ": targ,
            "mu64": mu[:, None], "gsc64": np.broadcast_to(gsc.T, (S, GS)),
            "ones_f": np.ones((BS, 1), np.float32),
            "ones64": np.ones((S, 1), np.float32),
            "dmy8": dmy8, "bfreg": bfreg,
        }
        blobv = np.zeros((BS, BLOB_COLS), np.float32)
        for nm, r, cc in _FIELDS:
            blobv[0:r, _OFF[nm]:_OFF[nm] + cc] = vals[nm]
        in_maps.append({"blob": blobv})
    return in_maps


def kernel(data, mu_param, alpha, beta, _trace=False):
    if "nc" not in _CACHE:
        _CACHE["nc"] = _build_program()
    nc = _CACHE["nc"]
    in_maps = _host_inputs(np.asarray(data), mu_param, alpha, beta)
    res = run_bass_kernel_spmd(nc, in_maps, list(range(8)), trace=_trace)
    t1 = sum(float(r["outd"][0, 0]) for r in res.results)
    t2 = sum(float(r["outd"][0, 1]) for r in res.results)
    out = np.float32(t1 - t2)
    if _trace:
        return np.asarray(out), res
    return np.asarray(out)
